# revision 1
# baseline (speedup 1.0000x reference)
"""Trainium2 Bass kernel for nn_BioSimulator.

Math: out[b,h,w] = clip(2 * sum_n Bw[b,n] * exp(-((px-vx[n])^2+(py-vy[n])^2)
                        * deg2pix^2 / (2*sigma_px[b,n]^2)), 0, 1)

px varies only along w and py only along h, so the Gaussian separates:
    exp(-(dx^2+dy^2)*c) = exp(-dx^2*c) * exp(-dy^2*c)
and the sum over points becomes a matmul over the point axis:
    out[b].T = Gx^T @ (2*Bw*Gy)        (transposed-output formulation)

Sharding: batch (2) x point-shards (4): each of the 8 cores handles one batch
and 256 of the N=1024 points (two 128-point partition tiles, accumulated in
PSUM across the two tiles).  Each core emits an unclipped partial
[2(wc),128(wp),256(h)]; the host sums the 4 shards per batch, transposes, and
clips.

Device per core:
  - DMA in pp[128,4] (stimulation + sigma scale, one column per point-tile)
    and sqd0/sqd1[128,512] = -0.5*[((xs-vx)*d2p)^2 | ((ys-vy)*d2p)^2].
  - Neuron math on [128,2] tiles (sigmoid via 1/(1+exp(-x)) so only the
    exp_and_others ACT table set is ever loaded; no sqrt needed because
    max(sqrt(v),1)^2 == max(v,1) for v>=0).
  - Per point-tile: one fused Exp [128,512] -> Gx|Gy in fp32r (rounded fp32:
    full-rate matmuls when the moving dim is >=256, near-fp32 accuracy,
    fp32 exponent range), scale Gy by 2*Bw, two PSUM-accumulating matmuls
    (w-chunks), copy out via DVE/ACT in parallel, DMA on both HWDGE rings.
"""

import numpy as np

import concourse.bass as bass
import concourse.bacc as bacc
import concourse.mybir as mybir
from concourse import tile
from concourse.bass_utils import run_bass_kernel_spmd

N_CORES = 8
NSHARDS = 4        # point shards per batch
PPC = 256          # points per core
NPT = 128          # points per partition tile
B = 2
H = W = 256

SPREAD = 0.000675
R2S = 0.5
SLOPE = 19152642.5
HALF = 1.057e-07
RHEO = 2.39e-05
FREQ = 300.0
PW = 0.00017
I_SCALE = 8e-05

F32 = mybir.dt.float32
F16 = mybir.dt.float16
F32R = mybir.dt.float32r
ALU = mybir.AluOpType
ACT = mybir.ActivationFunctionType

_NC = None


def _build_nc():
    nc = bacc.Bacc(None, target_bir_lowering=False, debug=False,
                   num_devices=N_CORES)
    pp = nc.dram_tensor("pp", [NPT, 4], F32, kind="ExternalInput")
    sqd0 = nc.dram_tensor("sqd0", [NPT, 2 * W], F32, kind="ExternalInput")
    sqd1 = nc.dram_tensor("sqd1", [NPT, 2 * W], F32, kind="ExternalInput")
    partial = nc.dram_tensor("partial", [2, 128, W], F32, kind="ExternalOutput")

    with tile.TileContext(nc) as tc:
        with (
            tc.tile_pool(name="const", bufs=1) as cpool,
            tc.tile_pool(name="work", bufs=2) as wpool,
            tc.tile_pool(name="obuf", bufs=2) as opool,
            tc.tile_pool(name="psum", bufs=2, space="PSUM") as psum,
        ):
            ppt = cpool.tile([NPT, 4], F32)
            nc.sync.dma_start(ppt[:], pp[:])
            sqdt = [cpool.tile([NPT, 2 * W], F32, tag=f"sqd{p}", name=f"sqdt{p}") for p in range(2)]
            nc.sync.dma_start(sqdt[0][:], sqd0[:])
            nc.sync.dma_start(sqdt[1][:], sqd1[:])

            # Cold-start absorber: a throwaway matmul on data that is ready
            # long before the real ones (PE is idle until ~3.7us otherwise),
            # so the real matmuls run at the warm clock with no LDW stall.
            wdum = cpool.tile([NPT, 2], F32)
            nc.vector.memset(wdum[:], 0.0)
            psd = psum.tile([2, 64], F32, tag="psd", name="psd", bufs=1)
            nc.tensor.matmul(psd[:], wdum[:], sqdt[0][:, 0:64], start=True, stop=True)
            # Table-load anchor: the exp table set loads before the first
            # ACTIVATE; give it one with no input-DMA dependency so the
            # ~1.3us load overlaps the input DMA instead of following it.
            dume = cpool.tile([NPT, 2], F32)
            nc.scalar.activation(dume[:], wdum[:], ACT.Exp)

            # -- Bw = sigmoid(SLOPE*(Q-HALF)).  The relu inside Q is replaced
            # exactly by clamping Bw from below: 1/(1+exp(A(s-t0)+C)) is
            # increasing in s and equals BW0 = 1/(1+e^C) at the threshold, so
            # Bw = max(1/(1+exp(A*s + (C-A*t0))), BW0).  The affine rides the
            # activation (bias memset at t=0), so the e-exp waits only on the
            # input DMA -- no DVE op ahead of it.
            bbias = cpool.tile([NPT, 1], F32)
            nc.vector.memset(bbias[:], float(SLOPE * (HALF + PW * FREQ * RHEO)))
            e = cpool.tile([NPT, 2], F32)
            nc.scalar.activation(
                e[:], ppt[:, 0:2], ACT.Exp,
                bias=bbias[:], scale=float(-SLOPE * PW * FREQ * I_SCALE),
            )
            ope = cpool.tile([NPT, 2], F32)
            nc.vector.tensor_scalar(ope[:], e[:], 1.0, None, ALU.add)
            bwu = cpool.tile([NPT, 2], F32)
            nc.vector.reciprocal(bwu[:], ope[:])
            bw = cpool.tile([NPT, 2], F32)
            nc.vector.tensor_scalar(
                bw[:], bwu[:], float(1.0 / (1.0 + np.exp(SLOPE * HALF))), None,
                ALU.max,
            )

            # -- negc = 1/max(sigma_px^2, 1); sigma_px^2 = stim*minv2sc comes
            # pre-scaled from the host (constant per-point factor), and the
            # -0.5 is baked into sqd, so exp(sqd * negc) is the Gaussian.
            v = cpool.tile([NPT, 2], F32)
            nc.vector.tensor_scalar(v[:], ppt[:, 2:4], 1.0, None, ALU.max)
            negc = cpool.tile([NPT, 2], F32)
            nc.vector.reciprocal(negc[:], v[:])

            # Per point-tile Gaussians; PSUM accumulates over the two tiles.
            pss = [psum.tile([128, W], F32, tag=f"ps{wc}", name=f"ps{wc}") for wc in range(2)]
            for p in range(2):
                gxy = wpool.tile([NPT, 2 * W], F32R, tag="gxy")
                nc.scalar.activation(
                    gxy[:], sqdt[p][:], ACT.Exp, scale=negc[:, p:p + 1],
                )
                gys = wpool.tile([NPT, W], F32R, tag="gys")
                nc.vector.tensor_scalar(
                    gys[:], gxy[:, W:2 * W], bw[:, p:p + 1], 2.0, ALU.mult, ALU.mult
                )
                # Transposed formulation: stationary = Gx chunk (ready before
                # gys), moving = gys; LDWEIGHTS stays off the critical path.
                for wc in range(2):
                    nc.tensor.matmul(
                        pss[wc][:],
                        gxy[:, wc * 128:(wc + 1) * 128],
                        gys[:],
                        start=(p == 0), stop=(p == 1),
                    )
            for wc in range(2):
                ob = opool.tile([128, W], F32)
                # Copies split across DVE and ACT so they run concurrently;
                # each DMA goes out on its issuer's HWDGE ring.
                if wc == 0:
                    nc.vector.tensor_copy(ob[:], pss[wc][:])
                    nc.sync.dma_start(partial[wc], ob[:])
                else:
                    nc.scalar.copy(ob[:], pss[wc][:])
                    nc.scalar.dma_start(partial[wc], ob[:])
    nc.compile()
    return nc


def _get_nc():
    global _NC
    if _NC is None:
        _NC = _build_nc()
    return _NC


def make_in_maps(stimulation, vx, vy, M, px, py, idx):
    stimulation = np.asarray(stimulation, dtype=np.float32)
    vx = np.asarray(vx, dtype=np.float32)
    vy = np.asarray(vy, dtype=np.float32)
    M = np.asarray(M, dtype=np.float32)
    px = np.asarray(px, dtype=np.float32)
    py = np.asarray(py, dtype=np.float32)
    idx = np.asarray(idx)

    fov = np.float32(px.max())
    deg2pix = np.float32(W) / (fov * np.float32(2.0))
    xs = px[0, :]            # px[h,w] = xs[w]
    ys = py[:, 0]            # py[h,w] = ys[h]
    flat = stimulation.reshape(B, -1)[:, idx]          # [B, N]
    minv2sc = (I_SCALE / SPREAD) * (R2S * deg2pix / M) ** 2  # [N]

    def sqd_for(sl):
        dx = (xs[None, :] - vx[sl, None]) * deg2pix    # [NPT, W]
        dy = (ys[None, :] - vy[sl, None]) * deg2pix    # [NPT, H]
        # -0.5 baked in: exponent = sqd * (1/max(sigma_px^2, 1))
        out = np.concatenate([dx * dx, dy * dy], axis=1) * np.float32(-0.5)
        return np.ascontiguousarray(out, dtype=np.float32)

    in_maps = []
    for c in range(N_CORES):
        b, s = divmod(c, NSHARDS)
        sl0 = slice(s * PPC, s * PPC + NPT)
        sl1 = slice(s * PPC + NPT, (s + 1) * PPC)
        pp = np.zeros((NPT, 4), np.float32)
        pp[:, 0] = flat[b, sl0]
        pp[:, 1] = flat[b, sl1]
        pp[:, 2] = flat[b, sl0] * minv2sc[sl0]
        pp[:, 3] = flat[b, sl1] * minv2sc[sl1]
        in_maps.append({
            "pp": pp,
            "sqd0": sqd_for(sl0),
            "sqd1": sqd_for(sl1),
        })
    return in_maps


def combine(results):
    acc = np.zeros((B, H, W), np.float32)
    for c, r in enumerate(results):
        b = c // NSHARDS
        # device emits out'[wc, wp, h]; out[b, h, wc*128+wp] = out'[...]
        p = r["partial"]
        acc[b] += p.transpose(2, 0, 1).reshape(H, W)
    return np.clip(acc, 0.0, 1.0)[:, None, :, :].astype(np.float32)


def kernel(stimulation, vx, vy, M, px, py, idx):
    nc = _get_nc()
    in_maps = make_in_maps(stimulation, vx, vy, M, px, py, idx)
    res = run_bass_kernel_spmd(nc, in_maps, list(range(N_CORES)))
    return combine(res.results)



# revision 9
# speedup vs baseline: 1.6607x; 1.6607x over previous
"""Trainium2 Bass kernel for nn_BioSimulator.

Math: out[b,h,w] = clip(2 * sum_n Bw[b,n] * exp(-((px-vx[n])^2+(py-vy[n])^2)
                        * deg2pix^2 / (2*sigma_px[b,n]^2)), 0, 1)

px varies only along w and py only along h, so the Gaussian separates:
    exp(-(dx^2+dy^2)*c) = exp(-dx^2*c) * exp(-dy^2*c)
and the sum over points becomes a matmul over the point axis:
    out[b].T = Gx^T @ (2*Bw*Gy)        (transposed-output formulation)

Sharding: batch (2) x point-shards (4): each of the 8 cores handles one batch
and 256 of the N=1024 points (two 128-point partition tiles, accumulated in
PSUM across the two tiles).  Each core emits an unclipped bf16 partial
[128(wp), 2(wc) x 256(h)]; the host sums the 4 shards per batch and clips.

Per-core device program (everything per-point is baked into the input table
on the host, so the device has NO scalar prep and NO small DMAs):
  - ONE input DMA: sq [128, 1024] bf16 -- per point-tile p and point row n:
      sq[n, 512p +   w] = negc * ((xs[w]-vx)*d2p)^2               (w half)
      sq[n, 512p+256+h] = negc * ((ys[h]-vy)*d2p)^2 + ln(2*Bw)    (h half)
    with negc = -0.5/max(sigma_px^2, 1).  The exponent is bf16: its rounding
    is RELATIVE to the exponent value, so the Gaussian error stays ~0.4% where
    it matters (full-pipeline rel_l2 vs reference: 1.3e-3).
  - ACT: exp per tile [128,512] bf16 -> f32r (Gx | 2Bw*Gy fused by the bake).
  - PE: 4 matmuls (stationary = Gx 128-column chunk, moving = 2Bw*Gy, full
    rate f32r since moving dim is 256) accumulating into 2 PSUM banks.
  - DVE/ACT copy the two banks into one bf16 SBUF tile (cast = half the
    output bytes), running concurrently.
  - Output via a PREPARED SWDGE scatter (dma_scatter_add prepare_only +
    trigger_dma): descriptors are generated on the idle Pool engine early in
    the kernel, so the post-compute tail is just trigger + transfer + sem --
    skipping the ~1.3us HWDGE-issue + DGE-delay chain a plain dma_start pays.
    Scatter indices (identity) are built on-device with memset+iota.
"""

import numpy as np
import ml_dtypes

import concourse.bass as bass
import concourse.bacc as bacc
import concourse.mybir as mybir
from concourse import tile
from concourse.bass_utils import run_bass_kernel_spmd

N_CORES = 8
NSHARDS = 4        # point shards per batch
PPC = 256          # points per core
NPT = 128          # points per partition tile
B = 2
H = W = 256

SPREAD = 0.000675
R2S = 0.5
SLOPE = 19152642.5
HALF = 1.057e-07
RHEO = 2.39e-05
FREQ = 300.0
PW = 0.00017
I_SCALE = 8e-05

F32 = mybir.dt.float32
F32R = mybir.dt.float32r
BF16 = mybir.dt.bfloat16
I16 = mybir.dt.int16
ACT = mybir.ActivationFunctionType

_NC = None


def _build_nc():
    nc = bacc.Bacc(None, target_bir_lowering=False, debug=False,
                   num_devices=N_CORES)
    sq = nc.dram_tensor("sq", [NPT, 1024], BF16, kind="ExternalInput")
    # partial[wc*128 + p, h]: one 512-byte row per output-w value.  Rows are
    # written by the SWDGE scatter; >512B scatter rows double-add
    # nondeterministically on HW (exceeds a single SDMA packet), 512B is safe.
    partial = nc.dram_tensor("partial", [2 * NPT, W], BF16,
                             kind="ExternalOutput")

    with tile.TileContext(nc) as tc:
        with (
            tc.tile_pool(name="const", bufs=1) as cpool,
            tc.tile_pool(name="psum", bufs=2, space="PSUM") as psum,
        ):
            # Table-load anchor: a throwaway exp with no DMA dependency so
            # the ~1.3us exp-table load overlaps the input DMA latency.
            wdum = cpool.tile([NPT, 2], F32)
            nc.vector.memset(wdum[:], 0.0)
            dume = cpool.tile([NPT, 2], F32)
            nc.scalar.activation(dume[:], wdum[:], ACT.Exp)

            # Identity scatter indices ([16, 16] int16 wrapped: index i lives
            # at [i%16, i//16] with value i; unused partitions stay 0).
            idxt = cpool.tile([NPT, 16], I16)
            nc.gpsimd.memset(idxt[:], 0)
            nc.gpsimd.iota(idxt[0:16, :], pattern=[[16, 16]], base=0,
                           channel_multiplier=1)

            # ob[p, wc, h]: scatter token i reads [i%128, i//128, :], so
            # tokens 0-127 are the pss0 rows and 128-255 the pss1 rows.
            ob = cpool.tile([NPT, 2, W], BF16)

            sqt = cpool.tile([NPT, 1024], BF16)
            nc.sync.dma_start(sqt[:], sq[:])

            gxy = [cpool.tile([NPT, 512], F32R, name=f"gxy{p}")
                   for p in range(2)]
            for p in range(2):
                nc.scalar.activation(gxy[p][:], sqt[:, 512 * p:512 * (p + 1)],
                                     ACT.Exp)

            pss = [psum.tile([NPT, W], F32, tag=f"ps{wc}", name=f"ps{wc}")
                   for wc in range(2)]
            for p in range(2):
                for wc in range(2):
                    nc.tensor.matmul(
                        pss[wc][:],
                        gxy[p][:, wc * 128:(wc + 1) * 128],
                        gxy[p][:, 256:512],
                        start=(p == 0), stop=(p == 1),
                    )
            # Copies split across DVE and ACT so they run concurrently.
            nc.vector.tensor_copy(ob[:, 0, :], pss[0][:])
            nc.scalar.copy(ob[:, 1, :], pss[1][:])
            # Prepared scatter: must be EMITTED after ob's writers so Tile
            # records the RAW edges (and defers them to the trigger), but it
            # still EXECUTES early -- its only sync dep is the idx tile, so
            # descriptor generation runs on the idle Pool engine during the
            # input-DMA latency.  The post-compute tail is then just
            # trigger + transfer + completion sem.
            dma_sem = nc.alloc_semaphore("swdge_dma")
            nc.gpsimd.dma_scatter_add(
                partial[:], ob[:], idxt[:],
                2 * NPT, 2 * NPT, W,
                prepare_only=True, sem=dma_sem,
            )
            nc.gpsimd.trigger_dma(count=None)
    nc.compile()
    return nc


def _get_nc():
    global _NC
    if _NC is None:
        _NC = _build_nc()
    return _NC


def make_in_maps(stimulation, vx, vy, M, px, py, idx):
    stimulation = np.asarray(stimulation, dtype=np.float64)
    vx = np.asarray(vx, dtype=np.float64)
    vy = np.asarray(vy, dtype=np.float64)
    M = np.asarray(M, dtype=np.float64)
    px = np.asarray(px, dtype=np.float64)
    py = np.asarray(py, dtype=np.float64)
    idx = np.asarray(idx)

    fov = px.max()
    d2p = W / (fov * 2.0)
    xs = px[0, :]            # px[h,w] = xs[w]
    ys = py[:, 0]            # py[h,w] = ys[h]
    flat = stimulation.reshape(B, -1)[:, idx]          # [B, N]
    I = flat * I_SCALE
    Bw = 1.0 / (1.0 + np.exp(-SLOPE * (np.maximum(I - RHEO, 0.0) * PW * FREQ
                                       - HALF)))
    sig2px = np.maximum((I / SPREAD) * (R2S * d2p / M[None, :]) ** 2, 1.0)
    negc = -0.5 / sig2px                               # [B, N]
    ln2bw = np.log(2.0 * Bw)                           # [B, N]

    in_maps = []
    for c in range(N_CORES):
        b, s = divmod(c, NSHARDS)
        sq = np.empty((NPT, 1024), np.float64)
        for p in range(2):
            sl = slice(s * PPC + NPT * p, s * PPC + NPT * (p + 1))
            nc_ = negc[b, sl][:, None]
            sq[:, 512 * p:512 * p + 256] = (
                nc_ * ((xs[None, :] - vx[sl, None]) * d2p) ** 2)
            sq[:, 512 * p + 256:512 * p + 512] = (
                nc_ * ((ys[None, :] - vy[sl, None]) * d2p) ** 2
                + ln2bw[b, sl][:, None])
        in_maps.append({"sq": sq.astype(ml_dtypes.bfloat16)})
    return in_maps


def combine(results):
    acc = np.zeros((B, H, W), np.float32)
    for c, r in enumerate(results):
        b = c // NSHARDS
        # partial[w, h] = contribution to out[b, h, w]
        acc[b] += np.asarray(r["partial"]).astype(np.float32).T
    return np.clip(acc, 0.0, 1.0)[:, None, :, :].astype(np.float32)


def kernel(stimulation, vx, vy, M, px, py, idx):
    nc = _get_nc()
    in_maps = make_in_maps(stimulation, vx, vy, M, px, py, idx)
    res = run_bass_kernel_spmd(nc, in_maps, list(range(N_CORES)))
    return combine(res.results)


# revision 12
# speedup vs baseline: 1.7561x; 1.0575x over previous
"""Trainium2 Bass kernel for nn_BioSimulator.

Math: out[b,h,w] = clip(2 * sum_n Bw[b,n] * exp(-((px-vx[n])^2+(py-vy[n])^2)
                        * deg2pix^2 / (2*sigma_px[b,n]^2)), 0, 1)

px varies only along w and py only along h, so the Gaussian separates:
    exp(-(dx^2+dy^2)*c) = exp(-dx^2*c) * exp(-dy^2*c)
and the sum over points becomes a matmul over the point axis:
    out[b].T = Gx^T @ (2*Bw*Gy)        (transposed-output formulation)

Sharding: batch (2) x point-shards (4), with the 1024 points SORTED by their
w-pixel position and sharded into quartiles.  sigma_px <= 2.01 px for this
parameterization, so a Gaussian's support is < +-14 px; each sorted quartile
(width <= 93 padded px) then fits a single 128-column w-window, meaning each
core touches only 128 of the 256 output columns: ONE PSUM bank, one matmul
per 128-point tile, one output copy.  Columns outside every window receive
contributions < 1e-9 in the reference and are exactly 0 here.

Per-core device program (everything per-point is baked into the input table
on the host, so the device has NO scalar prep and NO small DMAs):
  - ONE input DMA: sq [128, 768] bf16 -- per point-tile p and point row n:
      sq[n, 384p + j]       = negc * ((xs[w0+j]-vx)*d2p)^2             (Gx)
      sq[n, 384p + 128 + h] = negc * ((ys[h]-vy)*d2p)^2 + ln(2*Bw)     (Gy)
    with negc = -0.5/max(sigma_px^2, 1).  The exponent is bf16: its rounding
    is RELATIVE to the exponent value, so the Gaussian error stays ~0.4%
    where it matters (full-pipeline rel_l2 vs reference: ~1.3e-3).
  - ACT: exp per tile [128,384] bf16 -> f32r (Gx | 2Bw*Gy fused by the bake).
  - PE: 2 matmuls (stationary = Gx window, moving = 2Bw*Gy, full rate f32r
    since the moving dim is 256) accumulating into one PSUM bank.
  - DVE+ACT each copy half the bank into a bf16 SBUF tile, concurrently.
  - Output via a PREPARED SWDGE scatter (dma_scatter_add prepare_only +
    trigger_dma): descriptors are generated on the idle Pool engine early in
    the kernel, so the post-compute tail is just trigger + transfer + sem --
    skipping the ~1.3us HWDGE-issue + DGE-delay chain a plain dma_start pays.
    Scatter rows are 512 bytes: >512B rows double-add nondeterministically on
    HW (they exceed a single SDMA packet).  Indices (identity) are built
    on-device with memset+iota.
"""

import numpy as np
import ml_dtypes

import concourse.bass as bass
import concourse.bacc as bacc
import concourse.mybir as mybir
from concourse import tile
from concourse.bass_utils import run_bass_kernel_spmd

N_CORES = 8
NSHARDS = 4        # point shards per batch
PPC = 256          # points per core
NPT = 128          # points per partition tile
B = 2
H = W = 256
WIN = 128          # output w-window per core
MARGIN = 14.0      # px; exp(-0.5*(14/2.01)^2) ~ 3e-11, below bf16 noise

SPREAD = 0.000675
R2S = 0.5
SLOPE = 19152642.5
HALF = 1.057e-07
RHEO = 2.39e-05
FREQ = 300.0
PW = 0.00017
I_SCALE = 8e-05

F32 = mybir.dt.float32
F32R = mybir.dt.float32r
BF16 = mybir.dt.bfloat16
I16 = mybir.dt.int16
ACT = mybir.ActivationFunctionType

_NC = None


def _build_nc():
    nc = bacc.Bacc(None, target_bir_lowering=False, debug=False,
                   num_devices=N_CORES)
    sq = nc.dram_tensor("sq", [NPT, 2 * (WIN + H)], BF16,
                        kind="ExternalInput")
    # partial[p, h]: contribution to out[b, h, w0 + p]; one 512-byte row per
    # window column, written by the SWDGE scatter.
    partial = nc.dram_tensor("partial", [WIN, H], BF16, kind="ExternalOutput")

    with tile.TileContext(nc) as tc:
        with (
            tc.tile_pool(name="const", bufs=1) as cpool,
            tc.tile_pool(name="psum", bufs=1, space="PSUM") as psum,
        ):
            # Identity scatter indices ([16, 8] int16 wrapped: index i lives
            # at [i%16, i//16] with value i; unused partitions stay 0).
            idxt = cpool.tile([NPT, 8], I16)
            nc.gpsimd.memset(idxt[:], 0)
            nc.gpsimd.iota(idxt[0:16, :], pattern=[[16, 8]], base=0,
                           channel_multiplier=1)

            ob = cpool.tile([WIN, 1, H], BF16)

            sqt = cpool.tile([NPT, 2 * (WIN + H)], BF16)
            nc.sync.dma_start(sqt[:], sq[:])

            # exp goes straight on the ACT queue: the auto-inserted exp-table
            # load (~1.3us, no data deps) runs during the input-DMA latency
            # and is the critical-path head.
            gxy = [cpool.tile([NPT, WIN + H], F32R, name=f"gxy{p}")
                   for p in range(2)]
            for p in range(2):
                nc.scalar.activation(
                    gxy[p][:], sqt[:, (WIN + H) * p:(WIN + H) * (p + 1)],
                    ACT.Exp)

            pss = psum.tile([WIN, H], F32)
            for p in range(2):
                nc.tensor.matmul(
                    pss[:], gxy[p][:, 0:WIN], gxy[p][:, WIN:WIN + H],
                    start=(p == 0), stop=(p == 1),
                )
            # Copy halves split across DVE and ACT so they run concurrently.
            nc.vector.tensor_copy(ob[:, 0, 0:H // 2], pss[:, 0:H // 2])
            nc.scalar.copy(ob[:, 0, H // 2:H], pss[:, H // 2:H])
            # Prepared scatter: EMITTED after ob's writers so Tile records the
            # RAW edges (and defers them to the trigger), but it EXECUTES
            # early -- its only sync dep is the idx tile, so descriptor
            # generation runs on the idle Pool engine during the input-DMA
            # latency.  The post-compute tail is then just trigger + transfer.
            dma_sem = nc.alloc_semaphore("swdge_dma")
            nc.gpsimd.dma_scatter_add(
                partial[:], ob[:], idxt[:],
                WIN, WIN, H,
                prepare_only=True, sem=dma_sem,
            )
            nc.gpsimd.trigger_dma(count=None)
    nc.compile()
    return nc


def _get_nc():
    global _NC
    if _NC is None:
        _NC = _build_nc()
    return _NC


def _plan(vx, px):
    """Sort points by w-pixel position; pick each quartile's 128-col window."""
    fov = px.max()
    d2p = W / (fov * 2.0)
    wx = (vx + fov) * d2p
    order = np.argsort(wx)
    w0s = []
    for s in range(NSHARDS):
        ws = wx[order[s * PPC:(s + 1) * PPC]]
        lo = int(np.floor(ws.min() - MARGIN))
        hi = int(np.ceil(ws.max() + MARGIN)) + 1
        assert hi - lo <= WIN, (lo, hi)
        w0 = min(max(lo, 0), W - WIN)
        assert w0 <= lo and hi <= w0 + WIN, (lo, hi, w0)
        w0s.append(w0)
    return order, w0s, d2p


def make_in_maps(stimulation, vx, vy, M, px, py, idx):
    stimulation = np.asarray(stimulation, dtype=np.float64)
    vx = np.asarray(vx, dtype=np.float64)
    vy = np.asarray(vy, dtype=np.float64)
    M = np.asarray(M, dtype=np.float64)
    px = np.asarray(px, dtype=np.float64)
    py = np.asarray(py, dtype=np.float64)
    idx = np.asarray(idx)

    order, w0s, d2p = _plan(vx, px)
    xs = px[0, :]            # px[h,w] = xs[w]
    ys = py[:, 0]            # py[h,w] = ys[h]
    flat = stimulation.reshape(B, -1)[:, idx]          # [B, N]
    I = flat * I_SCALE
    Bw = 1.0 / (1.0 + np.exp(-SLOPE * (np.maximum(I - RHEO, 0.0) * PW * FREQ
                                       - HALF)))
    sig2px = np.maximum((I / SPREAD) * (R2S * d2p / M[None, :]) ** 2, 1.0)
    negc = -0.5 / sig2px                               # [B, N]
    ln2bw = np.log(2.0 * Bw)                           # [B, N]

    CW = WIN + H
    in_maps = []
    for c in range(N_CORES):
        b, s = divmod(c, NSHARDS)
        w0 = w0s[s]
        sq = np.empty((NPT, 2 * CW), np.float64)
        for p in range(2):
            sel = order[s * PPC + NPT * p:s * PPC + NPT * (p + 1)]
            nc_ = negc[b, sel][:, None]
            sq[:, CW * p:CW * p + WIN] = (
                nc_ * ((xs[None, w0:w0 + WIN] - vx[sel, None]) * d2p) ** 2)
            sq[:, CW * p + WIN:CW * (p + 1)] = (
                nc_ * ((ys[None, :] - vy[sel, None]) * d2p) ** 2
                + ln2bw[b, sel][:, None])
        in_maps.append({"sq": sq.astype(ml_dtypes.bfloat16)})
    return in_maps


def combine(results, w0s):
    acc = np.zeros((B, H, W), np.float32)
    for c, r in enumerate(results):
        b, s = divmod(c, NSHARDS)
        w0 = w0s[s]
        # partial[p, h] = contribution to out[b, h, w0 + p]
        acc[b, :, w0:w0 + WIN] += np.asarray(r["partial"]).astype(np.float32).T
    return np.clip(acc, 0.0, 1.0)[:, None, :, :].astype(np.float32)


def kernel(stimulation, vx, vy, M, px, py, idx):
    nc = _get_nc()
    in_maps = make_in_maps(stimulation, vx, vy, M, px, py, idx)
    _, w0s, _ = _plan(np.asarray(vx, np.float64), np.asarray(px, np.float64))
    res = run_bass_kernel_spmd(nc, in_maps, list(range(N_CORES)))
    return combine(res.results, w0s)


# revision 18
# speedup vs baseline: 2.0420x; 1.1628x over previous
"""Trainium2 Bass kernel for nn_BioSimulator.

Math: out[b,h,w] = clip(2 * sum_n Bw[b,n] * exp(-((px-vx[n])^2+(py-vy[n])^2)
                        * deg2pix^2 / (2*sigma_px[b,n]^2)), 0, 1)

px varies only along w and py only along h, so the Gaussian separates:
    exp(-(dx^2+dy^2)*c) = exp(-dx^2*c) * exp(-dy^2*c)
and the sum over points becomes a matmul over the point axis (transposed
output: stationary = Gx window, moving = 2Bw*Gy window).

sigma_px <= 2.01 px for this parameterization, so a Gaussian's support is
< +-14 px around its center (vx, vy).  That makes the problem windowable:

  - The 1024 points are SORTED by w-pixel position and sharded into
    quartiles (batch 2 x quartile 4 = 8 cores).  Every quartile spans
    <= 95 padded pixels -> each core touches a single 128-column w-window.
  - Within a quartile, points are sorted by h-pixel position and split into
    two 128-point tiles.  Every such tile spans <= 141 padded pixels -> each
    tile touches a 144-column h-window.

Each tile is then ONE matmul [Gx-window 128]^T @ [2Bw*Gy-window 144] into its
own PSUM bank, and the host pastes the two per-tile banks at their h-offsets
while summing shards (overlapping h-windows just add, exactly like shards).
Pixels outside every window receive < 3e-11 per point in the reference and
are exactly 0 here (the output is later clipped to [0,1] anyway).

Per-core device program (all per-point math is baked into the input table on
the host, so the device has NO scalar prep and NO small DMAs):
  - ONE input DMA: sq [128, 544] bf16 -- per tile p (cols 272p..272p+271),
    point row n (h0 = the tile's h-window start):
      sq[n, 272p + j]       = negc * ((xs[w0+j]-vx)*d2p)^2             (Gx)
      sq[n, 272p + 128 + j] = negc * ((ys[h0+j]-vy)*d2p)^2 + ln(2*Bw)  (Gy)
    with negc = -0.5/max(sigma_px^2, 1).  The exponent is bf16: its rounding
    is RELATIVE to the exponent value, so the Gaussian error stays ~0.4%
    where it matters (full-pipeline rel_l2 vs reference: ~1.3e-3).
  - ACT: exp per tile [128, 272] bf16 -> bf16 (Gx | 2Bw*Gy fused by the
    bake).  The auto-inserted exp-table load (~1.3us, no data deps) runs
    during the input-DMA latency and is the critical-path head.
  - PE: one bf16 matmul per tile (bf16 keeps full rate at 144 moving
    columns, unlike f32r) into that tile's PSUM bank.
  - DVE copies bank0 while tile1 is still in ACT/PE; ACT copies bank1.
    Separate banks per copy: PSUM reads of one tile serialize (destructive-
    read hazard), separate banks run concurrently.
  - Output via a PREPARED SWDGE scatter (dma_scatter_add prepare_only +
    trigger_dma): descriptors are generated on the idle Pool engine early in
    the kernel, so the post-compute tail is just trigger + transfer + sem --
    skipping the ~1.3us HWDGE-issue + DGE-delay chain a plain dma_start
    pays.  Scatter rows are 288 bytes (>512B rows double-add
    nondeterministically on HW -- they exceed a single SDMA packet).
    Indices (identity) are built on-device with memset+iota.
"""

import numpy as np
import ml_dtypes

import concourse.bass as bass
import concourse.bacc as bacc
import concourse.mybir as mybir
from concourse import tile
from concourse.bass_utils import run_bass_kernel_spmd

N_CORES = 8
NSHARDS = 4        # point shards (w-quartiles) per batch
PPC = 256          # points per core
NPT = 128          # points per partition tile
B = 2
H = W = 256
WIN = 128          # output w-window per core
HWIN = 144         # output h-window per point tile
MARGIN = 14.0      # px; exp(-0.5*(14/2.01)^2) ~ 3e-11, below bf16 noise
CW = WIN + HWIN    # table columns per tile

SPREAD = 0.000675
R2S = 0.5
SLOPE = 19152642.5
HALF = 1.057e-07
RHEO = 2.39e-05
FREQ = 300.0
PW = 0.00017
I_SCALE = 8e-05

F32 = mybir.dt.float32
BF16 = mybir.dt.bfloat16
I16 = mybir.dt.int16
ACT = mybir.ActivationFunctionType

_NC = None


def _build_nc():
    nc = bacc.Bacc(None, target_bir_lowering=False, debug=False,
                   num_devices=N_CORES)
    sq = nc.dram_tensor("sq", [NPT, 2 * CW], BF16, kind="ExternalInput")
    # partial[t*128 + p, j] (j < HWIN): contribution to out[b, h0[t]+j, w0+p].
    # Rows are padded to 256 columns: the scatter's row stride must be a
    # multiple of 256 bytes.
    partial = nc.dram_tensor("partial", [2 * WIN, H], BF16,
                             kind="ExternalOutput")

    with tile.TileContext(nc) as tc:
        with (
            tc.tile_pool(name="const", bufs=1) as cpool,
            tc.tile_pool(name="psum", bufs=2, space="PSUM") as psum,
        ):
            # Identity scatter indices ([16, 16] int16 wrapped: index i lives
            # at [i%16, i//16] with value i; unused partitions stay 0).
            idxt = cpool.tile([NPT, 16], I16)
            nc.gpsimd.memset(idxt[:], 0)
            nc.gpsimd.iota(idxt[0:16, :], pattern=[[16, 16]], base=0,
                           channel_multiplier=1)

            # ob[p, t, j]: scatter token i reads [i%128, i//128, :], so
            # tokens 0-127 are the tile-0 bank rows and 128-255 tile-1's.
            ob = cpool.tile([WIN, 2, HWIN], BF16)

            sqt = cpool.tile([NPT, 2 * CW], BF16)
            nc.sync.dma_start(sqt[:], sq[:])

            gxy = [cpool.tile([NPT, CW], BF16, name=f"gxy{p}")
                   for p in range(2)]
            psh = [psum.tile([WIN, H], F32, tag=f"ps{p}", name=f"ps{p}")
                   for p in range(2)]
            for p in range(2):
                nc.scalar.activation(gxy[p][:], sqt[:, CW * p:CW * (p + 1)],
                                     ACT.Exp)
                nc.tensor.matmul(
                    psh[p][:, 0:HWIN],
                    gxy[p][:, 0:WIN],
                    gxy[p][:, WIN:CW],
                    start=True, stop=True,
                )
            # tile-0 copy (DVE) overlaps tile-1's exp+matmul; ACT then
            # copies tile-1's bank.
            nc.vector.tensor_copy(ob[:, 0, :], psh[0][:, 0:HWIN])
            nc.scalar.copy(ob[:, 1, :], psh[1][:, 0:HWIN])
            # Prepared scatter: EMITTED after ob's writers so Tile records
            # the RAW edges (and defers them to the trigger), but it EXECUTES
            # early -- its only sync dep is the idx tile, so descriptor
            # generation runs on the idle Pool engine during the input-DMA
            # latency.  The post-compute tail is then just trigger+transfer.
            dma_sem = nc.alloc_semaphore("swdge_dma")
            nc.gpsimd.dma_scatter_add(
                partial[:, 0:HWIN], ob[:], idxt[:],
                2 * WIN, 2 * WIN, HWIN, elem_step=H,
                prepare_only=True, sem=dma_sem,
            )
            nc.gpsimd.trigger_dma(count=None)
    nc.compile()
    return nc


def _get_nc():
    global _NC
    if _NC is None:
        _NC = _build_nc()
    return _NC


def _plan(vx, vy, px):
    """Sort points by w-pixel into quartiles; vy-sort tiles inside each;
    pick each core's w-window and each tile's h-window."""
    fov = px.max()
    d2p = W / (fov * 2.0)
    wx = (vx + fov) * d2p
    wy = (vy + fov) * d2p
    order = np.argsort(wx)

    def window(pos, width):
        # Support clipped to the screen: off-screen Gaussian mass has no
        # output pixels, so only [0, W) needs covering.
        lo = max(0, int(np.floor(pos.min() - MARGIN)))
        hi = min(W, int(np.ceil(pos.max() + MARGIN)) + 1)
        assert hi - lo <= width, (lo, hi, width)
        start = min(lo, W - width)
        assert start <= lo and hi <= start + width, (lo, hi, start)
        return start

    sels, w0s, h0s = [], [], []
    for s in range(NSHARDS):
        q = order[s * PPC:(s + 1) * PPC]
        w0s.append(window(wx[q], WIN))
        q = q[np.argsort(wy[q])]
        tiles, th0 = [], []
        for t in range(2):
            sel = q[t * NPT:(t + 1) * NPT]
            tiles.append(sel)
            th0.append(window(wy[sel], HWIN))
        sels.append(tiles)
        h0s.append(th0)
    return sels, w0s, h0s, d2p


def make_in_maps(stimulation, vx, vy, M, px, py, idx):
    stimulation = np.asarray(stimulation, dtype=np.float64)
    vx = np.asarray(vx, dtype=np.float64)
    vy = np.asarray(vy, dtype=np.float64)
    M = np.asarray(M, dtype=np.float64)
    px = np.asarray(px, dtype=np.float64)
    py = np.asarray(py, dtype=np.float64)
    idx = np.asarray(idx)

    sels, w0s, h0s, d2p = _plan(vx, vy, px)
    xs = px[0, :]            # px[h,w] = xs[w]
    ys = py[:, 0]            # py[h,w] = ys[h]
    flat = stimulation.reshape(B, -1)[:, idx]          # [B, N]
    I = flat * I_SCALE
    Bw = 1.0 / (1.0 + np.exp(-SLOPE * (np.maximum(I - RHEO, 0.0) * PW * FREQ
                                       - HALF)))
    sig2px = np.maximum((I / SPREAD) * (R2S * d2p / M[None, :]) ** 2, 1.0)
    negc = -0.5 / sig2px                               # [B, N]
    ln2bw = np.log(2.0 * Bw)                           # [B, N]

    in_maps = []
    for c in range(N_CORES):
        b, s = divmod(c, NSHARDS)
        w0 = w0s[s]
        sq = np.empty((NPT, 2 * CW), np.float64)
        for p in range(2):
            sel = sels[s][p]
            h0 = h0s[s][p]
            nc_ = negc[b, sel][:, None]
            sq[:, CW * p:CW * p + WIN] = (
                nc_ * ((xs[None, w0:w0 + WIN] - vx[sel, None]) * d2p) ** 2)
            sq[:, CW * p + WIN:CW * (p + 1)] = (
                nc_ * ((ys[None, h0:h0 + HWIN] - vy[sel, None]) * d2p) ** 2
                + ln2bw[b, sel][:, None])
        in_maps.append({"sq": sq.astype(ml_dtypes.bfloat16)})
    return in_maps


def combine(results, w0s, h0s):
    acc = np.zeros((B, H, W), np.float32)
    for c, r in enumerate(results):
        b, s = divmod(c, NSHARDS)
        w0 = w0s[s]
        part = np.asarray(r["partial"])[:, 0:HWIN].astype(np.float32)
        for t in range(2):
            h0 = h0s[s][t]
            # partial[t*128+p, j] -> out[b, h0+j, w0+p]
            acc[b, h0:h0 + HWIN, w0:w0 + WIN] += part[t * WIN:(t + 1) * WIN].T
    return np.clip(acc, 0.0, 1.0)[:, None, :, :].astype(np.float32)


def kernel(stimulation, vx, vy, M, px, py, idx):
    nc = _get_nc()
    in_maps = make_in_maps(stimulation, vx, vy, M, px, py, idx)
    _, w0s, h0s, _ = _plan(np.asarray(vx, np.float64),
                           np.asarray(vy, np.float64),
                           np.asarray(px, np.float64))
    res = run_bass_kernel_spmd(nc, in_maps, list(range(N_CORES)))
    return combine(res.results, w0s, h0s)


# revision 25
# speedup vs baseline: 2.1391x; 1.0476x over previous
"""Trainium2 Bass kernel for nn_BioSimulator.

Math: out[b,h,w] = clip(2 * sum_n Bw[b,n] * exp(-((px-vx[n])^2+(py-vy[n])^2)
                        * deg2pix^2 / (2*sigma_px[b,n]^2)), 0, 1)

px varies only along w and py only along h, so the Gaussian separates:
    exp(-(dx^2+dy^2)*c) = exp(-dx^2*c) * exp(-dy^2*c)
and the sum over points becomes a matmul over the point axis (transposed
output: stationary = Gx window, moving = 2Bw*Gy window).

sigma_px <= 2.01 px for this parameterization, so a Gaussian's support is
< +-14 px around its center (vx, vy).  That makes the problem windowable:

  - The 1024 points are SORTED by w-pixel position and sharded into
    quartiles (batch 2 x quartile 4 = 8 cores).  Every quartile spans
    <= 95 padded pixels -> each core touches a single 128-column w-window.
  - Within a quartile, points are sorted by h-pixel position and split into
    two 128-point tiles.  Every such tile spans <= 141 padded pixels -> each
    tile touches a 144-column h-window.

Each tile is then ONE matmul [Gx-window 128]^T @ [2Bw*Gy-window 144] into its
own PSUM bank, and the host pastes the two per-tile banks at their h-offsets
while summing shards (overlapping h-windows just add, exactly like shards).
Pixels outside every window receive < 3e-11 per point in the reference and
are exactly 0 here (the output is later clipped to [0,1] anyway).

Per-core device program (all per-point math is baked into the input table on
the host, so the device has NO scalar prep and NO small DMAs):
  - ONE input DMA: sq [128, 544] bf16 -- per tile p (cols 272p..272p+271),
    point row n (h0 = the tile's h-window start):
      sq[n, 272p + j]       = negc * ((xs[w0+j]-vx)*d2p)^2             (Gx)
      sq[n, 272p + 128 + j] = negc * ((ys[h0+j]-vy)*d2p)^2 + ln(2*Bw)  (Gy)
    with negc = -0.5/max(sigma_px^2, 1).  The exponent is bf16: its rounding
    is RELATIVE to the exponent value, so the Gaussian error stays ~0.4%
    where it matters (full-pipeline rel_l2 vs reference: ~1.3e-3).
  - ACT: exp per tile [128, 272] bf16 -> bf16 (Gx | 2Bw*Gy fused by the
    bake).  The auto-inserted exp-table load (~1.3us, no data deps) runs
    during the input-DMA latency and is the critical-path head.
  - PE: one bf16 matmul per tile (bf16 keeps full rate at 144 moving
    columns, unlike f32r) into that tile's PSUM bank.
  - DVE copies bank0 while tile1 is still in ACT/PE; ACT copies bank1.
    Separate banks per copy: PSUM reads of one tile serialize (destructive-
    read hazard), separate banks run concurrently.
  - Output via a PREPARED SWDGE scatter (dma_scatter_add prepare_only +
    trigger_dma): descriptors are generated on the idle Pool engine early in
    the kernel, so the post-compute tail is just trigger + transfer + sem --
    skipping the ~1.3us HWDGE-issue + DGE-delay chain a plain dma_start
    pays.  Scatter rows are 288 bytes (>512B rows double-add
    nondeterministically on HW -- they exceed a single SDMA packet).
    Indices (identity) are built on-device with memset+iota.
"""

import numpy as np
import ml_dtypes

import concourse.bass as bass
import concourse.bacc as bacc
import concourse.mybir as mybir
from concourse import tile
from concourse.bass_utils import run_bass_kernel_spmd

N_CORES = 8
NSHARDS = 4        # point shards (w-quartiles) per batch
PPC = 256          # points per core
NPT = 128          # points per partition tile
B = 2
H = W = 256
WIN = 96           # output w-window per core (every quartile spans <= 95)
HWIN = 144         # output h-window per point tile
MARGIN = 14.0      # px; exp(-0.5*(14/2.01)^2) ~ 3e-11, below bf16 noise
CW = WIN + HWIN    # table columns per tile
DUMP = 2 * WIN     # partial dump row for the unused scatter tokens

SPREAD = 0.000675
R2S = 0.5
SLOPE = 19152642.5
HALF = 1.057e-07
RHEO = 2.39e-05
FREQ = 300.0
PW = 0.00017
I_SCALE = 8e-05

F32 = mybir.dt.float32
BF16 = mybir.dt.bfloat16
I16 = mybir.dt.int16
ACT = mybir.ActivationFunctionType

_NC = None


def _build_nc():
    nc = bacc.Bacc(None, target_bir_lowering=False, debug=False,
                   num_devices=N_CORES)
    sq = nc.dram_tensor("sq", [NPT, 2 * CW], BF16, kind="ExternalInput")
    # partial[t*96 + p, j] (j < HWIN): contribution to out[b, h0[t]+j, w0+p].
    # Rows are padded to 256 columns (the scatter's row stride must be a
    # multiple of 256 bytes); row DUMP swallows the unused scatter tokens.
    partial = nc.dram_tensor("partial", [2 * WIN + 1, H], BF16,
                             kind="ExternalOutput")

    with tile.TileContext(nc) as tc:
        with (
            tc.tile_pool(name="const", bufs=1) as cpool,
            tc.tile_pool(name="psum", bufs=2, space="PSUM") as psum,
        ):
            # Scatter indices, [16, 16] int16 wrapped: token i lives at
            # [i%16, i//16].  Tokens 0-95 -> rows 0-95 (tile-0 bank),
            # 128-223 -> rows 96-191 (tile-1 bank); tokens 96-127 read
            # unused ob partitions and go to the DUMP row (interior tokens
            # may not be -1); trailing tokens 224-255 are -1 (ignored).
            idxt = cpool.tile([NPT, 16], I16)
            nc.gpsimd.memset(idxt[:], 0)
            nc.gpsimd.iota(idxt[0:16, 0:6], pattern=[[16, 6]], base=0,
                           channel_multiplier=1)
            nc.gpsimd.memset(idxt[0:16, 6:8], DUMP)
            nc.gpsimd.iota(idxt[0:16, 8:14], pattern=[[16, 6]], base=WIN,
                           channel_multiplier=1)
            nc.gpsimd.memset(idxt[0:16, 14:16], -1)

            # ob[p, t, j]: scatter token i reads [i%128, i//128, :]; only
            # partitions 0-95 carry bank rows, the rest feed the DUMP row
            # (zeroed so the reads are defined).
            ob = cpool.tile([NPT, 2, HWIN], BF16)
            nc.gpsimd.memset(ob[WIN:NPT, :, :], 0)

            sqt = cpool.tile([NPT, 2 * CW], BF16)
            nc.sync.dma_start(sqt[:], sq[:])

            gxy = [cpool.tile([NPT, CW], BF16, name=f"gxy{p}")
                   for p in range(2)]
            # Tile 0: one matmul into one bank, copied by DVE while tile 1
            # is still in ACT/PE.  Tile 1: two matmuls into two separate
            # banks (h sub-halves, same total PE cycles) so its copy can be
            # split across DVE and ACT -- PSUM reads of a single tile are
            # serialized by the framework (destructive-read hazard), two
            # banks run concurrently.  The split (85/59) balances DVE
            # (1.04ns/col + 125) against ACT (0.83ns/col + 164).
            HA = 85
            ps0 = psum.tile([WIN, H], F32, tag="ps0", name="ps0")
            ps1a = psum.tile([WIN, H], F32, tag="ps1a", name="ps1a")
            ps1b = psum.tile([WIN, H], F32, tag="ps1b", name="ps1b")
            nc.scalar.activation(gxy[0][:], sqt[:, 0:CW], ACT.Exp)
            nc.tensor.matmul(ps0[:, 0:HWIN], gxy[0][:, 0:WIN],
                             gxy[0][:, WIN:CW], start=True, stop=True)
            nc.scalar.activation(gxy[1][:], sqt[:, CW:2 * CW], ACT.Exp)
            nc.tensor.matmul(ps1a[:, 0:HA], gxy[1][:, 0:WIN],
                             gxy[1][:, WIN:WIN + HA], start=True, stop=True)
            nc.tensor.matmul(ps1b[:, 0:HWIN - HA], gxy[1][:, 0:WIN],
                             gxy[1][:, WIN + HA:CW], start=True, stop=True)
            nc.vector.tensor_copy(ob[0:WIN, 0, :], ps0[:, 0:HWIN])
            nc.vector.tensor_copy(ob[0:WIN, 1, 0:HA], ps1a[:, 0:HA])
            nc.scalar.copy(ob[0:WIN, 1, HA:HWIN], ps1b[:, 0:HWIN - HA])
            # Prepared scatter: EMITTED after ob's writers so Tile records
            # the RAW edges (and defers them to the trigger), but it EXECUTES
            # early -- its only sync dep is the idx tile, so descriptor
            # generation runs on the idle Pool engine during the input-DMA
            # latency.  The post-compute tail is then just trigger+transfer.
            dma_sem = nc.alloc_semaphore("swdge_dma")
            nc.gpsimd.dma_scatter_add(
                partial[:, 0:HWIN], ob[:], idxt[:],
                2 * NPT, 2 * NPT - 32, HWIN, elem_step=H,
                prepare_only=True, sem=dma_sem,
            )
            nc.gpsimd.trigger_dma(count=None)
    nc.compile()
    return nc


def _get_nc():
    global _NC
    if _NC is None:
        _NC = _build_nc()
    return _NC


def _plan(vx, vy, px):
    """Sort points by w-pixel into quartiles; vy-sort tiles inside each;
    pick each core's w-window and each tile's h-window."""
    fov = px.max()
    d2p = W / (fov * 2.0)
    wx = (vx + fov) * d2p
    wy = (vy + fov) * d2p
    order = np.argsort(wx)

    def window(pos, width):
        # Support clipped to the screen: off-screen Gaussian mass has no
        # output pixels, so only [0, W) needs covering.
        lo = max(0, int(np.floor(pos.min() - MARGIN)))
        hi = min(W, int(np.ceil(pos.max() + MARGIN)) + 1)
        assert hi - lo <= width, (lo, hi, width)
        start = min(lo, W - width)
        assert start <= lo and hi <= start + width, (lo, hi, start)
        return start

    sels, w0s, h0s = [], [], []
    for s in range(NSHARDS):
        q = order[s * PPC:(s + 1) * PPC]
        w0s.append(window(wx[q], WIN))
        q = q[np.argsort(wy[q])]
        tiles, th0 = [], []
        for t in range(2):
            sel = q[t * NPT:(t + 1) * NPT]
            tiles.append(sel)
            th0.append(window(wy[sel], HWIN))
        sels.append(tiles)
        h0s.append(th0)
    return sels, w0s, h0s, d2p


def make_in_maps(stimulation, vx, vy, M, px, py, idx):
    stimulation = np.asarray(stimulation, dtype=np.float64)
    vx = np.asarray(vx, dtype=np.float64)
    vy = np.asarray(vy, dtype=np.float64)
    M = np.asarray(M, dtype=np.float64)
    px = np.asarray(px, dtype=np.float64)
    py = np.asarray(py, dtype=np.float64)
    idx = np.asarray(idx)

    sels, w0s, h0s, d2p = _plan(vx, vy, px)
    xs = px[0, :]            # px[h,w] = xs[w]
    ys = py[:, 0]            # py[h,w] = ys[h]
    flat = stimulation.reshape(B, -1)[:, idx]          # [B, N]
    I = flat * I_SCALE
    Bw = 1.0 / (1.0 + np.exp(-SLOPE * (np.maximum(I - RHEO, 0.0) * PW * FREQ
                                       - HALF)))
    sig2px = np.maximum((I / SPREAD) * (R2S * d2p / M[None, :]) ** 2, 1.0)
    negc = -0.5 / sig2px                               # [B, N]
    ln2bw = np.log(2.0 * Bw)                           # [B, N]

    in_maps = []
    for c in range(N_CORES):
        b, s = divmod(c, NSHARDS)
        w0 = w0s[s]
        sq = np.empty((NPT, 2 * CW), np.float64)
        for p in range(2):
            sel = sels[s][p]
            h0 = h0s[s][p]
            nc_ = negc[b, sel][:, None]
            sq[:, CW * p:CW * p + WIN] = (
                nc_ * ((xs[None, w0:w0 + WIN] - vx[sel, None]) * d2p) ** 2)
            sq[:, CW * p + WIN:CW * (p + 1)] = (
                nc_ * ((ys[None, h0:h0 + HWIN] - vy[sel, None]) * d2p) ** 2
                + ln2bw[b, sel][:, None])
        in_maps.append({"sq": sq.astype(ml_dtypes.bfloat16)})
    return in_maps


def combine(results, w0s, h0s):
    acc = np.zeros((B, H, W), np.float32)
    for c, r in enumerate(results):
        b, s = divmod(c, NSHARDS)
        w0 = w0s[s]
        part = np.asarray(r["partial"])[:, 0:HWIN].astype(np.float32)
        for t in range(2):
            h0 = h0s[s][t]
            # partial[t*128+p, j] -> out[b, h0+j, w0+p]
            acc[b, h0:h0 + HWIN, w0:w0 + WIN] += part[t * WIN:(t + 1) * WIN].T
    return np.clip(acc, 0.0, 1.0)[:, None, :, :].astype(np.float32)


def kernel(stimulation, vx, vy, M, px, py, idx):
    nc = _get_nc()
    in_maps = make_in_maps(stimulation, vx, vy, M, px, py, idx)
    _, w0s, h0s, _ = _plan(np.asarray(vx, np.float64),
                           np.asarray(vy, np.float64),
                           np.asarray(px, np.float64))
    res = run_bass_kernel_spmd(nc, in_maps, list(range(N_CORES)))
    return combine(res.results, w0s, h0s)
